# revision 13
# baseline (speedup 1.0000x reference)
"""Multi-head attention block (B=4, S=2048, D=1024, H=16, DH=64) on 8 trn2 cores.

Sharding: tensor-parallel over heads (2 groups of 8) x data-parallel over batch (4).
Core c handles batch c//2, heads (c%2)*8 .. +8. Each core computes a partial
output projection (its 8 heads' contribution to cat @ W0); the host sums the
two partials per batch and adds b0.

Per-core kernel layout (all tensors per this core's batch/head-group):
  xT   [1024, 2048] fp32   x transposed (host-prepped)
  wq/wk/wv [1024, 512] fp32,  w0 [512, 1024] fp16
  qT/kT stored as [128(e of head-pair), 512(s-block)] fp16 tiles
  v stored s-major with a ones column per head: [128(s), 8*65] fp16
  scoresT[key, q] = kT.T @ qT per 128-key chunk -> exp on ACT (scale=1/8)
  PV: ctxT+denom = [v_h | 1].T @ expT accumulated over key chunks (M=65)
  normalize: recip(denom) -> partition_broadcast -> multiply -> catT fp16
  out = catT.T @ w0 accumulated over the 4 head-pairs -> psum -> HBM
"""

import os
import sys

for _p in ("/opt/trn_rl_repo",):
    if _p not in sys.path and os.path.isdir(_p):
        sys.path.insert(0, _p)

import numpy as np

import concourse.bass as bass
import concourse.bacc as bacc_mod
import concourse.mybir as mybir
import concourse.tile as tile
import bass_rust
from concourse.vector_clock import ScopedClock

B, S, D, H, DH = 4, 2048, 1024, 16, 64
NCORES = 8
HL = 8            # heads per core
NP = HL // 2      # head pairs per core
E = HL * DH       # 512 local cat width
QB = 512          # q block (columns per attention block)
NQB = S // QB     # 4
KC = 128          # key chunk
NKC = S // KC     # 16
DC = 128          # d (contraction) chunk for projections
NDC = D // DC     # 8
F32 = mybir.dt.float32
F32R = mybir.dt.float32r
F16 = mybir.dt.float16
EXPSCALE = 1.0 / np.sqrt(DH)

_MAXW = 1


def _patched_drain_and_barrier(self, tick_clock, wait_clock):
    """Walrus codegen only supports one sync-wait per CTRL instruction; Tile's
    stock exit drain piles every outstanding processor's sem wait onto a single
    drain. Split them across nops (same engine => program order preserved)."""
    probe = self.nc.sync.nop()
    wait_clock.add_sem_waits(probe.ins, ScopedClock({None: tick_clock.global_clock}))
    si = probe.ins.sync_info
    waits = list(si.on_wait) if si is not None and si.on_wait else []
    if len(waits) > _MAXW:
        probe.ins.sync_info = bass_rust.SyncInfo(on_wait=waits[:_MAXW], on_update=[])
        for i in range(_MAXW, len(waits), _MAXW):
            extra = self.nc.sync.nop()
            extra.ins.sync_info = bass_rust.SyncInfo(
                on_wait=waits[i : i + _MAXW], on_update=[]
            )
    self.nc.sync.drain()
    self.nc.all_engine_barrier()
    popped = self.nc._tile_sem_poison_stack.pop()
    assert popped is self._sem_poison
    self.nc.clear_and_free_semaphores(list(self.sems.allocated().values()))
    self.nc.all_engine_barrier()


tile.TileContext._drain_and_barrier = _patched_drain_and_barrier


def build_nc(debug=False):
    nc = bacc_mod.Bacc()
    xT = nc.dram_tensor("xT", [D, S], F32R, kind="ExternalInput")
    wq = nc.dram_tensor("wq", [D, E], F32R, kind="ExternalInput")
    wk = nc.dram_tensor("wk", [D, E], F32R, kind="ExternalInput")
    wv = nc.dram_tensor("wv", [D, E], F32R, kind="ExternalInput")
    bqk = nc.dram_tensor("bqk", [128, 2 * NP], F32, kind="ExternalInput")
    bvr = nc.dram_tensor("bvr", [1, E], F32, kind="ExternalInput")
    w0 = nc.dram_tensor("w0", [E, D], F16, kind="ExternalInput")
    out = nc.dram_tensor("out", [S, D], F32, kind="ExternalOutput")
    dbg = {}
    if debug:
        for nm, shp in [("dbg_q", [128, QB]), ("dbg_k", [128, QB]),
                        ("dbg_v", [128, HL * 65]), ("dbg_e", [128, 1024]),
                        ("dbg_pv", [128, QB]), ("dbg_rb", [128, QB]),
                        ("dbg_cat", [128, S]), ("dbg_cat1", [128, S]),
                        ("dbg_cat2", [128, S]), ("dbg_cat3", [128, S])]:
            dbg[nm] = nc.dram_tensor(nm, shp, F32, kind="ExternalOutput")

    r = lambda ap: ap.bitcast(F32R)

    with tile.TileContext(nc) as tc:
        with (
            tc.tile_pool(name="plong", bufs=1) as plong,
            tc.tile_pool(name="pqkt", bufs=1) as pqkt,
            tc.tile_pool(name="pcat", bufs=1) as pcat,
            tc.tile_pool(name="pv", bufs=1) as pvpool,
            tc.tile_pool(name="pw0", bufs=1) as pw0,
        ):
            # ---- persistent small tiles ----
            bqkt = plong.tile([128, 2 * NP], F32, tag="bqkt", name="bqkt")
            nc.sync.dma_start(bqkt[:], bqk[:])
            bvrow = plong.tile([1, E], F32, tag="bvrow", name="bvrow")
            nc.sync.dma_start(bvrow[:], bvr[:])
            bvb = plong.tile([128, E], F32, tag="bvb", name="bvb")
            nc.gpsimd.partition_broadcast(bvb[:], bvrow[:])

            w0t = []
            for p in range(NP):
                t = pw0.tile([128, D], F16, tag=f"w0_{p}", name=f"w0_{p}")
                nc.sync.dma_start(t[:], w0[p * 128 : (p + 1) * 128, :])
                w0t.append(t)

            # catT tiles: one per head pair, [128 (2 heads x 64), S] fp16
            catT = [pcat.tile([128, S], F16, tag=f"cat{p}", name=f"cat{p}") for p in range(NP)]

            # v tiles (s-major, ones column per head), built in phase A
            vaug = [pvpool.tile([128, HL * 65], F16, tag=f"v{sc}", name=f"v{sc}") for sc in range(NKC)]

            # qT/kT tiles: [pair][sblock] -> [128, 512] fp16
            qt = [[None] * NQB for _ in range(NP)]
            kt = [[None] * NQB for _ in range(NP)]

            with (
                tc.tile_pool(name="pxt", bufs=8) as pxt,
                tc.tile_pool(name="pw", bufs=24) as pw,
                tc.tile_pool(name="psA", bufs=6, space="PSUM") as psA,
            ):
                xt = []
                for k in range(NDC):
                    t = pxt.tile([128, S], F32R, tag="xt", name="xt")
                    nc.sync.dma_start(t[:], xT[k * 128 : (k + 1) * 128, :])
                    xt.append(t)

                def load_w(dram):
                    ts = []
                    for k in range(NDC):
                        t = pw.tile([128, E], F32R, tag="w", name="w")
                        nc.sync.dma_start(t[:], dram[k * 128 : (k + 1) * 128, :])
                        ts.append(t)
                    return ts

                def proj_qk(wtiles, bias_col, dest, p):
                    # dest[p][sb] [128, QB] fp16 = (x @ W)[:, p*128:+128].T + bias
                    for sb in range(NQB):
                        ps = psA.tile([128, QB], F32, tag="ps", name="ps")
                        for k in range(NDC):
                            nc.tensor.matmul(
                                ps[:],
                                wtiles[k][:, p * 128 : (p + 1) * 128],
                                xt[k][:, sb * QB : (sb + 1) * QB],
                                start=(k == 0),
                                stop=(k == NDC - 1),
                            )
                        t = pqkt.tile([128, QB], F16, tag=f"qk{dest is kt}{p}{sb}", name="qkt")
                        nc.vector.tensor_scalar_add(
                            t[:], ps[:], bqkt[:, bias_col + p : bias_col + p + 1]
                        )
                        dest[p][sb] = t

                wq_t = load_w(wq)
                wk_t = load_w(wk)
                # pair 0 first so attention can start early
                proj_qk(wq_t, 0, qt, 0)
                proj_qk(wk_t, NP, kt, 0)

                wv_t = load_w(wv)
                for sc in range(NKC):
                    ps = psA.tile([128, E], F32, tag="ps", name="ps")
                    for k in range(NDC):
                        nc.tensor.matmul(
                            ps[:],
                            xt[k][:, sc * 128 : (sc + 1) * 128],
                            wv_t[k][:],
                            start=(k == 0),
                            stop=(k == NDC - 1),
                        )
                    va = vaug[sc]
                    nc.gpsimd.memset(
                        va[:].rearrange("p (h c) -> p h c", c=65)[:, :, 64:65], 1.0
                    )
                    nc.vector.tensor_add(
                        va[:].rearrange("p (h c) -> p h c", c=65)[:, :, 0:64],
                        ps[:].rearrange("p (h c) -> p h c", c=64),
                        bvb[:].rearrange("p (h c) -> p h c", c=64),
                    )

                for p in range(1, NP):
                    proj_qk(wq_t, 0, qt, p)
                    proj_qk(wk_t, NP, kt, p)

                if debug:
                    dq = pw.tile([128, QB], F32, tag="w", name="dbgq")
                    nc.vector.tensor_copy(dq[:], qt[0][0][:])
                    nc.sync.dma_start(dbg["dbg_q"][:], dq[:])
                    dk = pw.tile([128, QB], F32, tag="w", name="dbgk")
                    nc.vector.tensor_copy(dk[:], kt[0][0][:])
                    nc.sync.dma_start(dbg["dbg_k"][:], dk[:])
                    dv = pw.tile([128, HL * 65], F32, tag="w", name="dbgv")
                    nc.vector.tensor_copy(dv[:], vaug[0][:])
                    nc.sync.dma_start(dbg["dbg_v"][:], dv[:])

            # ---- attention ----
            with (
                tc.tile_pool(name="pexp", bufs=3) as pexp,
                tc.tile_pool(name="psm", bufs=4) as psm,
                tc.tile_pool(name="psCpv", bufs=2, space="PSUM") as psCpv,
                tc.tile_pool(name="psCs", bufs=1, space="PSUM") as psCs,
            ):
                for p in range(NP):
                    for qb in range(NQB):
                        qtile = qt[p][qb]
                        pv = [
                            psCpv.tile([65, QB], F32, tag=f"pv{sub}", name=f"pv{sub}") for sub in range(2)
                        ]
                        for kcg in range(NKC // 2):
                            psS = [
                                psCs.tile([128, 1024], F32, tag=f"psS{sub}", name=f"psS{sub}")
                                for sub in range(2)
                            ]
                            for j in range(2):
                                kc = kcg * 2 + j
                                ktile = kt[p][kc // 4]
                                kslice = slice((kc % 4) * 128, (kc % 4) * 128 + 128)
                                for sub in range(2):
                                    rows = slice(sub * 64, sub * 64 + 64)
                                    nc.tensor.matmul(
                                        psS[sub][:, j * QB : (j + 1) * QB],
                                        ktile[rows, kslice],
                                        qtile[rows, :],
                                        start=True,
                                        stop=True,
                                    )
                            et = [
                                pexp.tile([128, 1024], F16, tag=f"e{sub}", name=f"e{sub}")
                                for sub in range(2)
                            ]
                            for sub in range(2):
                                nc.scalar.activation(
                                    et[sub][:],
                                    psS[sub][:],
                                    mybir.ActivationFunctionType.Exp,
                                    scale=EXPSCALE,
                                )
                            if debug and p == 0 and qb == 0 and kcg == 0:
                                de = psm.tile([128, 1024], F32, tag="rb", name="dbge")
                                nc.vector.tensor_copy(de[:], et[0][:])
                                nc.sync.dma_start(dbg["dbg_e"][:], de[:])
                            for j in range(2):
                                kc = kcg * 2 + j
                                for sub in range(2):
                                    h = p * 2 + sub
                                    nc.tensor.matmul(
                                        pv[sub][:],
                                        vaug[kc][:, h * 65 : (h + 1) * 65],
                                        et[sub][:, j * QB : (j + 1) * QB],
                                        start=(kc == 0),
                                        stop=(kc == NKC - 1),
                                    )
                        # normalize: row 64 of pv = softmax denominator
                        if debug and p == 0 and qb == 0:
                            dpv = psm.tile([128, QB], F32, tag="rb", name="dbgpv")
                            nc.vector.tensor_copy(dpv[0:65, :], pv[0][:])
                            nc.sync.dma_start(dbg["dbg_pv"][0:65, :], dpv[0:65, :])
                        for sub in range(2):
                            # denominator row: ACT copy (psum, lane-aligned) ->
                            # DMA partition-shift -> reciprocal -> broadcast
                            dsb = psm.tile([128, QB], F32, tag="dsb", name="dsb")
                            nc.scalar.copy(dsb[64:65, :], pv[sub][64:65, :])
                            srow = psm.tile([1, QB], F32, tag="srow", name="srow")
                            nc.sync.dma_start(srow[:], dsb[64:65, :])
                            rrow = psm.tile([1, QB], F32, tag="rrow", name="rrow")
                            nc.vector.reciprocal_approx_fast(rrow[:], srow[:])
                            rb = psm.tile([64, QB], F32, tag="rb", name="rb")
                            nc.gpsimd.partition_broadcast(rb[:], rrow[:])
                            if debug and p == 0 and qb == 0 and sub == 0:
                                nc.sync.dma_start(dbg["dbg_rb"][0:64, :], rb[:])
                            if sub == 0:
                                nc.vector.tensor_mul(
                                    catT[p][0:64, qb * QB : (qb + 1) * QB],
                                    pv[sub][0:64, :],
                                    rb[:],
                                )
                            else:
                                tb = psm.tile([64, QB], F16, tag="tb", name="tb")
                                nc.vector.tensor_mul(tb[:], pv[sub][0:64, :], rb[:])
                                nc.sync.dma_start(
                                    catT[p][64:128, qb * QB : (qb + 1) * QB], tb[:]
                                )
                if debug:
                    for pp, dnm in enumerate(["dbg_cat", "dbg_cat1", "dbg_cat2", "dbg_cat3"]):
                        dct = pexp.tile([128, S], F32, tag="dcat", name="dcat")
                        nc.vector.tensor_copy(dct[:], catT[pp][:])
                        nc.sync.dma_start(dbg[dnm][:], dct[:])

            # ---- output projection ----
            with (
                tc.tile_pool(name="psD", bufs=4, space="PSUM") as psD,
                tc.tile_pool(name="pout", bufs=4) as pout,
            ):
                for sc in range(S // 128):
                    for db in range(D // QB):
                        ps = psD.tile([128, QB], F32, tag="po", name="po")
                        for p in range(NP):
                            nc.tensor.matmul(
                                ps[:],
                                catT[p][:, sc * 128 : (sc + 1) * 128],
                                w0t[p][:, db * QB : (db + 1) * QB],
                                start=(p == 0),
                                stop=(p == NP - 1),
                            )
                        ot = pout.tile([128, QB], F32, tag="ot", name="ot")
                        nc.vector.tensor_copy(ot[:], ps[:])
                        nc.sync.dma_start(
                            out[sc * 128 : (sc + 1) * 128, db * QB : (db + 1) * QB],
                            ot[:],
                        )
    nc.finalize()
    return nc


_NC_CACHE = None


def _get_nc():
    global _NC_CACHE
    if _NC_CACHE is None:
        _NC_CACHE = build_nc()
    return _NC_CACHE


def make_in_maps(x, Wq, bq, Wk, bk, Wv, bv, W0, b0):
    x = np.asarray(x, dtype=np.float32)
    in_maps = []
    xTb = [np.ascontiguousarray(x[b].T) for b in range(B)]
    for c in range(NCORES):
        b = c // 2
        h0 = (c % 2) * HL
        sl = slice(h0, h0 + HL)
        wq_c = np.ascontiguousarray(
            np.asarray(Wq[sl], np.float32).transpose(1, 0, 2).reshape(D, E)
        )
        wk_c = np.ascontiguousarray(
            np.asarray(Wk[sl], np.float32).transpose(1, 0, 2).reshape(D, E)
        )
        wv_c = np.ascontiguousarray(
            np.asarray(Wv[sl], np.float32).transpose(1, 0, 2).reshape(D, E)
        )
        bq_c = np.asarray(bq[sl], np.float32).reshape(E)
        bk_c = np.asarray(bk[sl], np.float32).reshape(E)
        bqk_c = np.empty((128, 2 * NP), np.float32)
        for g in range(NP):
            bqk_c[:, g] = bq_c[g * 128 : (g + 1) * 128]
            bqk_c[:, NP + g] = bk_c[g * 128 : (g + 1) * 128]
        bv_c = np.asarray(bv[sl], np.float32).reshape(1, E)
        w0_c = np.ascontiguousarray(
            np.asarray(W0[h0 * DH : (h0 + HL) * DH], np.float32).astype(np.float16)
        )
        in_maps.append(
            {
                "xT": xTb[b],
                "wq": wq_c,
                "wk": wk_c,
                "wv": wv_c,
                "bqk": bqk_c,
                "bvr": bv_c,
                "w0": w0_c,
            }
        )
    return in_maps


def combine(results, b0):
    out = np.empty((B, S, D), np.float32)
    for b in range(B):
        out[b] = results[2 * b]["out"] + results[2 * b + 1]["out"]
    out += np.asarray(b0, np.float32)[None, None, :]
    return out


def kernel(x, Wq, bq, Wk, bk, Wv, bv, W0, b0):
    from concourse.bass_utils import run_bass_kernel_spmd

    nc = _get_nc()
    in_maps = make_in_maps(x, Wq, bq, Wk, bk, Wv, bv, W0, b0)
    res = run_bass_kernel_spmd(nc, in_maps, core_ids=list(range(NCORES)))
    return combine(res.results, b0)
